# revision 18
# baseline (speedup 1.0000x reference)
"""BNAF (Block Neural Autoregressive Flow) Trainium2 kernel.

Self-contained: takes full inputs, shards batch across 8 NeuronCores,
returns (x_out (4096,64) f32, log_det (4096,) f32).

Math (per flow, per sample; weights batch-independent, precomputed on host):
  u  = wn1 @ x + b1                (H=8192; bias folded via ones-row, K=65)
  h  = tanh(u)
  y  = wn2 @ h + b2                (64)
  q_i = sum_k C[i,k] * h[i,k]^2    (block-diag weighted reduce, k=0..127)
  grad_i = log(S_i - q_i)          since exp(log(1-tanh^2)) = 1-h^2
  gated residual + log-det accumulation; inter-flow flip folded into
  host-side weight permutation.

Device layout per core: x kept transposed (64+1 ones row, 512), fp16
matmul operands (full PE rate), fp32 PSUM accumulation. The last flow's
q-path uses fp32 h and a scaled v = EK*(1-h^2) in fp16 to survive tanh
saturation (|u| reaches ~40). All Ln work is deferred behind the last
flow (explicit scheduler deps) so one activation-table switch suffices.
"""
import sys
import numpy as np

sys.path.insert(0, "/opt/trn_rl_repo")

N_FLOWS = 5
DIM = 64
NH = 128
BATCH = 4096
H = DIM * NH            # 8192
OD1 = NH                # 128 hidden units per dim-block
NCORES = 8
BL = BATCH // NCORES    # 512 samples per core
LNK = 10.5              # fp16 underflow-guard scale for last-flow v
EK = float(np.exp(LNK))
QFLOOR = 0.01           # floor for last-flow q' (tanh spline overshoot guard)

TB = 2                  # dim-blocks per L1 psum tile
L1BUFS = 3              # psum buffers for L1 tiles
MMDT = "bf16"           # matmul operand dtype: f16 | bf16 | f32r (bf16 is the only full-rate PE dtype on TRN2)


def _npdt():
    import ml_dtypes
    return {"f16": np.float16, "bf16": ml_dtypes.bfloat16,
            "f32r": np.float32}[MMDT]


def _masks(out_f, in_f, dim):
    od, idd = out_f // dim, in_f // dim
    mask_d = np.zeros((out_f, in_f), np.float32)
    mask_o = np.ones((out_f, in_f), np.float32)
    for i in range(dim):
        mask_d[i * od:(i + 1) * od, i * idd:(i + 1) * idd] = 1.0
        mask_o[i * od:(i + 1) * od, i * idd:] = 0.0
    return mask_d, mask_o


_MASK_D1, _MASK_O1 = _masks(H, DIM, DIM)
_MASK_D2, _MASK_O2 = _masks(DIM, H, DIM)


def _precompute_flow(W1, d1, b1, W2, d2, b2, perm):
    """Host weight transform for one flow with flip-permutation folded."""
    W1 = W1.astype(np.float32); W2 = W2.astype(np.float32)
    d1 = d1.astype(np.float32); d2 = d2.astype(np.float32)
    b1 = b1.astype(np.float32); b2 = b2.astype(np.float32)

    w1 = np.exp(W1) * _MASK_D1 + W1 * _MASK_O1
    wsn1 = np.sum(w1 * w1, axis=1, keepdims=True)
    wn1 = np.exp(d1) * w1 / np.sqrt(wsn1)             # (H, 64)
    g1 = (d1[:, 0] + W1[np.arange(H), np.repeat(np.arange(DIM), OD1)]
          - 0.5 * np.log(wsn1[:, 0])).reshape(DIM, OD1)

    w2 = np.exp(W2) * _MASK_D2 + W2 * _MASK_O2
    wsn2 = np.sum(w2 * w2, axis=1, keepdims=True)
    wn2 = np.exp(d2) * w2 / np.sqrt(wsn2)             # (64, H)
    cols = np.arange(DIM)[:, None] * OD1 + np.arange(OD1)[None, :]
    g2 = d2[:, 0][:, None] + W2[np.arange(DIM)[:, None], cols] \
        - 0.5 * np.log(wsn2[:, 0])[:, None]

    C = np.exp(g1 + g2).astype(np.float32)            # (64, 128)

    # fold permutation (device row j holds true dim perm[j])
    wn1p = wn1.reshape(DIM, OD1, DIM)[perm][:, :, perm].reshape(H, DIM)
    b1p = b1.reshape(DIM, OD1)[perm].reshape(H)
    wn2p = wn2.reshape(DIM, DIM, OD1)[perm][:, perm, :].reshape(DIM, H)
    b2p = b2[perm]
    Cp = C[perm]

    import ml_dtypes
    npdt = _npdt()
    C16 = Cp.astype(ml_dtypes.bfloat16)
    S = C16.astype(np.float32).sum(axis=1)            # (64,)

    w1aug = np.zeros((65, H), npdt)
    w1aug[:64, :] = wn1p.T.astype(npdt)
    w1aug[64, :] = b1p.astype(npdt)
    w2T = np.zeros((NH, DIM * DIM), npdt)             # (128, 4096)
    cw = np.zeros((NH, DIM * DIM), ml_dtypes.bfloat16)
    wn2b = wn2p.reshape(DIM, DIM, OD1)                # [m, i, k]
    for i in range(DIM):
        w2T[:, i * DIM:(i + 1) * DIM] = wn2b[:, i, :].T.astype(npdt)
        cw[:, i * DIM + i] = C16[i]
    return w1aug, w2T, cw, b2p, S


def precompute(W1, d1, b1, W2, d2, b2, gates):
    """Returns per-flow device constants + python scalars."""
    flows = []
    perm = np.arange(DIM)
    for f in range(N_FLOWS):
        last = f == N_FLOWS - 1
        w1aug, w2T, cw, b2p, S = _precompute_flow(
            W1[f], d1[f], b1[f], W2[f], d2[f], b2[f], perm)
        if not last:
            g = float(gates[f])
            s = float(1.0 / (1.0 + np.exp(-g)))
            ln_scale = float(-np.exp(g))
            ln_bias = (1.0 + np.exp(g) * S).astype(np.float32)
        else:
            s = 1.0
            ln_scale = float(np.exp(-LNK))
            ln_bias = np.zeros(DIM, np.float32)
        flows.append(dict(w1=w1aug, w2=w2T, cw=cw,
                          ybias=(s * b2p).astype(np.float32).reshape(DIM, 1),
                          s=s, ln_scale=ln_scale, ln_bias=ln_bias))
        perm = DIM - 1 - perm
    corr = sum(DIM * float(np.logaddexp(0.0, float(gates[f])))
               for f in range(N_FLOWS - 1))
    return flows, corr


def build_nc(flow_scalars):
    """Build the Bass program. flow_scalars: list of dicts with keys
    s, ln_scale (python floats baked into instructions)."""
    import concourse.mybir as mybir
    import concourse.tile as tile
    from concourse import bacc
    from concourse.bass import ts
    from concourse.tile_rust import add_dep_helper

    F16 = {"f16": mybir.dt.float16, "bf16": mybir.dt.bfloat16,
           "f32r": mybir.dt.float32r}[MMDT]
    BF16 = mybir.dt.bfloat16
    F32 = mybir.dt.float32
    AF = mybir.ActivationFunctionType
    ALU = mybir.AluOpType

    nc = bacc.Bacc("TRN2", target_bir_lowering=False, num_devices=NCORES)

    xb_in = nc.dram_tensor("xb0", [65, BL], F16, kind="ExternalInput")
    xm_in = nc.dram_tensor("xm0", [DIM, BL], F32, kind="ExternalInput")
    w1_in, w2_in, cw_in, yb_in = [], [], [], []
    for f in range(len(flow_scalars)):
        w1_in.append(nc.dram_tensor(f"w1_{f}", [65, H], F16, kind="ExternalInput"))
        w2_in.append(nc.dram_tensor(f"w2_{f}", [NH, DIM * DIM], F16, kind="ExternalInput"))
        cw_in.append(nc.dram_tensor(f"cw_{f}", [NH, DIM * DIM], BF16, kind="ExternalInput"))
        yb_in.append(nc.dram_tensor(f"yb_{f}", [DIM, 1], F32, kind="ExternalInput"))
    lnb_in = nc.dram_tensor("lnb", [DIM, len(flow_scalars)], F32, kind="ExternalInput")

    xout = nc.dram_tensor("xout", [DIM, BL], F32, kind="ExternalOutput")
    ldout = nc.dram_tensor("ldout", [1, BL], F32, kind="ExternalOutput")

    NT = (DIM + TB - 1) // TB

    with tile.TileContext(nc) as tc:
        with (
            tc.tile_pool(name="wp", bufs=2) as wp,
            tc.tile_pool(name="xp", bufs=2) as xp,
            tc.tile_pool(name="hp", bufs=3) as hp,
            tc.tile_pool(name="hqp", bufs=4) as hqp,
            tc.tile_pool(name="f4p", bufs=2) as f4p,
            tc.tile_pool(name="persist", bufs=1) as persist,
            tc.tile_pool(name="l1ps", bufs=L1BUFS, space="PSUM") as l1ps,
            tc.tile_pool(name="yps", bufs=1, space="PSUM") as yps,
            tc.tile_pool(name="qps", bufs=1, space="PSUM") as qps,
        ):
            xb = xp.tile([65, BL], F16, tag="xb")
            xm = xp.tile([DIM, BL], F32, tag="xm")
            nc.sync.dma_start(out=xb, in_=xb_in[:])
            nc.sync.dma_start(out=xm, in_=xm_in[:])

            qf = []
            last_flow_insts = []
            nfl = len(flow_scalars)
            for f in range(nfl):
                fs = flow_scalars[f]
                last = f == nfl - 1
                w1 = wp.tile([65, H], F16, tag="w1")
                w2 = wp.tile([NH, DIM * DIM], F16, tag="w2")
                cw = wp.tile([NH, DIM * DIM], BF16, tag="cw")
                yb = persist.tile([DIM, 1], F32, tag=f"yb{f}")
                nc.sync.dma_start(out=w1[:, :H // 2], in_=w1_in[f][:, :H // 2])
                nc.sync.dma_start(out=w1[:, H // 2:], in_=w1_in[f][:, H // 2:])
                nc.sync.dma_start(out=w2, in_=w2_in[f][:])
                nc.sync.dma_start(out=cw, in_=cw_in[f][:])
                nc.sync.dma_start(out=yb, in_=yb_in[f][:])

                Y = yps.tile([DIM, BL], F32, tag="y")
                Qt_full = qps.tile([NH, BL], F32, tag="q")
                Q = Qt_full[DIM:, :]

                for t in range(NT):
                    b0 = t * TB
                    nb = min(TB, DIM - b0)
                    w = nb * BL
                    ps = l1ps.tile([128, TB * BL], F32, tag="l1")
                    for j in range(nb):
                        blk = b0 + j
                        nc.tensor.matmul(ps[:, ts(j, BL)], w1[:, ts(blk, NH)],
                                         xb, start=True, stop=True)
                    if not last:
                        h = hp.tile([128, TB * BL], F16, tag="h")
                        nc.scalar.activation(h[:, :w], ps[:, :w], AF.Tanh)
                        hq = hqp.tile([128, TB * BL], BF16, tag="hq")
                        nc.vector.tensor_mul(hq[:, :w], h[:, :w], h[:, :w])
                        l2rhs = h
                    else:
                        h32 = hp.tile([128, TB * BL], F32, tag="h")
                        nc.scalar.activation(h32[:, :w], ps[:, :w], AF.Tanh)
                        hsq = hp.tile([128, TB * BL], F32, tag="hsq32")
                        nc.vector.tensor_mul(hsq[:, :w], h32[:, :w], h32[:, :w])
                        hq = hqp.tile([128, TB * BL], BF16, tag="hq")
                        nc.vector.tensor_scalar(hq[:, :w], hsq[:, :w], -EK, EK,
                                                op0=ALU.mult, op1=ALU.add)
                        h16 = f4p.tile([128, TB * BL], F16, tag="h16")
                        nc.vector.tensor_copy(h16[:, :w], h32[:, :w])
                        l2rhs = h16
                    for j in range(nb):
                        blk = b0 + j
                        nc.tensor.matmul(Y, w2[:, ts(blk, DIM)],
                                         l2rhs[:, ts(j, BL)],
                                         start=(blk == 0), stop=(blk == DIM - 1))
                        nc.tensor.matmul(Q, cw[:, ts(blk, DIM)],
                                         hq[:, ts(j, BL)],
                                         start=(blk == 0), stop=(blk == DIM - 1),
                                         tile_position=(0, DIM))

                # tail: evacuate q, gated residual
                qtf = persist.tile([NH, BL], F32, tag=f"qf{f}")
                qt = qtf[DIM:, :]
                if last:
                    qi = nc.vector.tensor_scalar_max(qt, Q, QFLOOR)
                else:
                    qi = nc.vector.tensor_copy(qt, Q)
                qf.append(qt)
                ys = xp.tile([DIM, BL], F32, tag="ys")
                yi = nc.scalar.activation(ys, Y, AF.Identity, bias=yb,
                                          scale=fs["s"])
                if last:
                    last_flow_insts = [qi.ins, yi.ins]
                    nc.sync.dma_start(out=xout[:], in_=ys)
                else:
                    xs = xp.tile([DIM, BL], F32, tag="xs")
                    nc.vector.tensor_scalar_mul(xs, xm, 1.0 - fs["s"])
                    # critical path: next flow's matmul input directly in f16
                    xb2 = xp.tile([65, BL], F16, tag="xb")
                    nc.vector.tensor_add(xb2[:DIM, :], ys, xs)
                    nc.vector.memset(xb2[DIM:65, :], 1.0)
                    # fp32 master for the next residual (off critical path)
                    xm2 = xp.tile([DIM, BL], F32, tag="xm")
                    nc.vector.tensor_add(xm2, ys, xs)
                    xm = xm2
                    xb = xb2

            # deferred Ln path, pinned behind the last flow so the
            # activation-table switches once instead of ten times
            lnbf = persist.tile([NH, nfl], F32, tag="lnb")
            lnb = lnbf[DIM:, :]
            nc.sync.dma_start(out=lnb, in_=lnb_in[:])
            onesf = persist.tile([NH, 1], F32, tag="ones")
            ones = onesf[DIM:, :]
            nc.vector.memset(ones, 1.0)
            ldsumf = xp.tile([NH, BL], F32, tag="ldsum")
            ldsum = ldsumf[DIM:, :]
            for f in range(nfl):
                lnff = xp.tile([NH, BL], F32, tag="lnf")
                lnf = lnff[DIM:, :]
                li = nc.scalar.activation(lnf, qf[f], AF.Ln,
                                          bias=lnb[:, f:f + 1],
                                          scale=flow_scalars[f]["ln_scale"])
                for dep in last_flow_insts:
                    add_dep_helper(li.ins, dep, sync=False,
                                   reason="defer Ln past last flow")
                if f == 0:
                    nc.vector.tensor_copy(ldsum, lnf)
                else:
                    ldsum2f = xp.tile([NH, BL], F32, tag="ldsum")
                    ldsum2 = ldsum2f[DIM:, :]
                    nc.vector.tensor_add(ldsum2, ldsum, lnf)
                    ldsum = ldsum2
            ldq = qps.tile([1, BL], F32, tag="q")
            nc.tensor.matmul(ldq, ones, ldsum, start=True, stop=True)
            ldt = xp.tile([1, BL], F32, tag="ldt")
            nc.vector.tensor_copy(ldt, ldq)
            nc.sync.dma_start(out=ldout[:], in_=ldt)

    nc.compile()
    return nc


_CACHE = {}


def _get_nc(flows):
    key = tuple((fl["s"], fl["ln_scale"]) for fl in flows)
    if key not in _CACHE:
        _CACHE[key] = build_nc(flows)
    return _CACHE[key]


def kernel(x, W1, d1, b1, W2, d2, b2, gates, _trace=False, _tmpdir=None):
    from concourse import bass_utils

    x = np.asarray(x); gates = np.asarray(gates)
    flows, corr = precompute(np.asarray(W1), np.asarray(d1), np.asarray(b1),
                             np.asarray(W2), np.asarray(d2), np.asarray(b2),
                             gates)
    nc = _get_nc(flows)

    lnb = np.stack([fl["ln_bias"] for fl in flows], axis=1).astype(np.float32)
    common = {}
    for f, fl in enumerate(flows):
        common[f"w1_{f}"] = fl["w1"]
        common[f"w2_{f}"] = fl["w2"]
        common[f"cw_{f}"] = fl["cw"]
        common[f"yb_{f}"] = fl["ybias"]
    common["lnb"] = lnb

    in_maps = []
    for c in range(NCORES):
        xs = x[c * BL:(c + 1) * BL].astype(np.float32)     # (512, 64)
        xT = np.ascontiguousarray(xs.T)                    # (64, 512)
        xb0 = np.ones((65, BL), _npdt())
        xb0[:DIM] = xT.astype(_npdt())
        m = dict(common)
        m["xb0"] = xb0
        m["xm0"] = xT
        in_maps.append(m)

    kw = {}
    if _trace:
        kw = dict(trace=True, tmpdir=_tmpdir)
    res = bass_utils.run_bass_kernel_spmd(nc, in_maps,
                                          core_ids=list(range(NCORES)), **kw)

    x_out = np.empty((BATCH, DIM), np.float32)
    ld_out = np.empty((BATCH,), np.float32)
    for c in range(NCORES):
        x_out[c * BL:(c + 1) * BL] = res.results[c]["xout"].T
        ld_out[c * BL:(c + 1) * BL] = res.results[c]["ldout"][0] - corr
    if _trace:
        return (x_out, ld_out), res
    return x_out, ld_out
